# revision 21
# baseline (speedup 1.0000x reference)
"""Trainium2 Bass kernel for nn_AttentionCompiled (dense transformer attention).

B=8, N=1024, C=768, H=12 heads, D=64. Per-head LayerNorm on q/k, softmax
attention, output projection. Pure data parallelism: one batch element per
NeuronCore, weights replicated, no collectives.

Math folding:
 - LN centering folded into Wq/Wk (CPU-side).
 - 1/sigma_q (with the 1/sqrt(D) scale) folded into q and 1/sigma_k into k via
   PE-broadcast matmul + DVE multiply, so Exp runs with scale=1 and both
   512-wide query blocks batch into one FD=1024 ACTIVATE.
 - 1/sigma = exp(-0.5*ln(scale*sumsq + eps)): ln and exp share ONE ACT table
   set (natural_log_exp_and_others), so per-pair stats interleave with
   attention exps with zero table reloads. build() pins that choice by
   emptying the single-function exp/ln sets in the table map (index-preserving
   — act_func_set_id indexes act_info.json).
 - Softmax denominators from an appended ones-column on V (row 64 of O^T).
 - |scores| <= 8 (Cauchy-Schwarz on LN'd vectors): exp needs no max-subtract.

Precision split: q/k generation and QK^T run in f32r (score exponents are
error-sensitive); the V path (x_bf16 @ Wv), P (exp output), O^T and the
projection run in bf16 with fp32 PSUM accumulation (gate is 2e-2).

Perf structure (v3):
 - Per-pair software pipeline: pair p's ACT-paced attention stream carries
   pair p+1's generation/stats/prescale matmuls (and, for pair 0, the whole
   V generation) as PE filler, so the PE instruction stream stays dense —
   idle PE windows re-engage the HAM clock gate (K=4/8, half clock), which
   is what capped v1/v2.
 - Loop order pr -> nb -> mt; score tile [128,1024] holds both heads
   (row-group concurrent QK); one Exp ACTIVATE per tile (FD=1024).
 - PSUM: 2x score tiles (4 banks) + 2x2 O^T accumulators [65,512] (4 banks,
   double-buffered so the next pair's PV never waits the epilogue copies).
 - Epilogue per (pr, nb): copy accumulators out, reciprocal_approx_fast
   (full-tile: the custom DVE op mishandles base_partition != 0), DMA
   row-broadcast via DRAM bounce, DVE normalize into bf16 O^T.
"""

import sys
import numpy as np
from contextlib import ExitStack

if "/opt/trn_rl_repo" not in sys.path:
    sys.path.insert(0, "/opt/trn_rl_repo")

import concourse.bass as bass
import concourse.bacc as bacc
import concourse.tile as tile
from concourse import mybir

F32 = mybir.dt.float32
F32R = mybir.dt.float32r
BF16 = mybir.dt.bfloat16

N = 1024
C = 768
H = 12
D = 64
NT = N // 128
CT = C // 128
NB = N // 512
NP = H // 2
EPS = 1e-5

USE_F32R = True


def _filtered_act_tables(arch):
    import concourse.hw_specs as hw_specs
    tabs = dict(hw_specs.get_activation_tables(arch))
    # empty them (never match) rather than delete: act_func_set_id is the
    # INDEX into this ordered dict and must stay aligned with act_info.json
    for k in ("exp_and_others", "natural_log", "exp_and_friends"):
        tabs[k] = set()
    return tabs


def build(use_f32r: bool = USE_F32R, compile_module: bool = True) -> bass.Bass:
    bacc.get_activation_tables = _filtered_act_tables
    nc = bacc.Bacc()

    xTb = nc.declare_dram_parameter("xTb", [C, N], BF16, isOutput=False)
    wq = nc.declare_dram_parameter("wqcT", [C, C], BF16, isOutput=False)
    wk = nc.declare_dram_parameter("wkcT", [C, C], BF16, isOutput=False)
    wv = nc.declare_dram_parameter("wvT", [C, C], BF16, isOutput=False)
    wp = nc.declare_dram_parameter("wpT", [C, C], BF16, isOutput=False)
    seg = nc.declare_dram_parameter("seg", [128, CT, H], BF16, isOutput=False)
    ind2d = nc.declare_dram_parameter("ind2", [2, 128], BF16, isOutput=False)
    out_ext = nc.declare_dram_parameter("out", [N, C], F32, isOutput=True)

    MMD = BF16

    def mm(ap):
        return ap

    with tile.TileContext(nc) as tc, ExitStack() as ctx:
        xo_pool = ctx.enter_context(tc.tile_pool(name="xo", bufs=6))
        persist = ctx.enter_context(tc.tile_pool(name="persist", bufs=1))
        vp_pool = ctx.enter_context(tc.tile_pool(name="vp", bufs=6))
        work = ctx.enter_context(tc.tile_pool(name="work", bufs=2))
        ptp = ctx.enter_context(tc.tile_pool(name="ptp", bufs=3))
        epi = ctx.enter_context(tc.tile_pool(name="epi", bufs=1))
        stp = ctx.enter_context(tc.tile_pool(name="stp", bufs=1))
        qkp = ctx.enter_context(tc.tile_pool(name="qkp", bufs=1))
        rows = ctx.enter_context(tc.tile_pool(name="rows", bufs=1))
        dramp = ctx.enter_context(tc.tile_pool(name="dramp", bufs=2, space="DRAM"))
        psS = ctx.enter_context(tc.tile_pool(name="psS", bufs=2, space="PSUM"))
        psG = ctx.enter_context(tc.tile_pool(name="psG", bufs=1, space="PSUM"))
        psOT = ctx.enter_context(tc.tile_pool(name="psOT", bufs=1, space="PSUM"))

        # ---- loads: spread across engine DMA queues so the prefix isn't
        # serialized on one queue, and interleave (wq[kc], xt[kc]) so the
        # first gen matmuls can start after the first c-tile lands ----
        def load_w(dram, nm, pool, tag, dt, eng, per_tile_tags=False):
            tiles = []
            for r in range(CT):
                t = pool.tile(
                    [128, C], dt, name=f"{nm}{r}",
                    tag=(f"{tag}{r}" if per_tile_tags else tag),
                )
                eng.dma_start(out=t, in_=mm(dram[128 * r:128 * (r + 1), :]))
                tiles.append(t)
            return tiles

        seg_sb = rows.tile([128, CT, H], MMD, name="seg", tag="seg")
        nc.sync.dma_start(out=seg_sb, in_=mm(seg[:, :, :]))
        ind2 = rows.tile([2, 128], MMD, name="ind2", tag="ind2")
        nc.sync.dma_start(out=ind2, in_=mm(ind2d[:, :]))

        xtb_sb = []
        wq_sb = load_w(wq, "wq", persist, "wq", BF16, nc.scalar, per_tile_tags=True)
        for r in range(CT):
            t = xo_pool.tile([128, N], BF16, name=f"xtb{r}", tag="xb")
            nc.sync.dma_start(out=t, in_=xTb[128 * r:128 * (r + 1), :])
            xtb_sb.append(t)
        wk_sb = load_w(wk, "wk", persist, "wk", BF16, nc.gpsimd, per_tile_tags=True)
        wv_sb = load_w(wv, "wv", vp_pool, "vp", BF16, nc.gpsimd)
        xt_sb = xtb_sb

        epsq2 = rows.tile([2, 1], F32, name="epsq2", tag="epsq2")
        nc.vector.memset(epsq2, float(D) * EPS)
        epsk2 = rows.tile([2, 1], F32, name="epsk2", tag="epsk2")
        nc.vector.memset(epsk2, EPS)

        qh_sb = [None] * NP
        kh_sb = [None] * NP
        sig_r = [None] * NP
        v_sb = [None] * NT
        ot_sb = [None] * CT

        def gen_qk(pr, which, pstag="g"):
            wt = wq_sb if which == "q" else wk_sb
            lst = qh_sb if which == "q" else kh_sb
            pool = psG if pstag == "g" else psS
            ps = pool.tile([128, N], F32, name=f"ps_{which}{pr}", tag=pstag)
            for kc in range(CT):
                for nb in range(NB):
                    nc.tensor.matmul(
                        ps[:, 512 * nb:512 * (nb + 1)],
                        lhsT=mm(wt[kc][:, 128 * pr:128 * (pr + 1)]),
                        rhs=mm(xt_sb[kc][:, 512 * nb:512 * (nb + 1)]),
                        start=(kc == 0),
                        stop=(kc == CT - 1),
                    )
            t = qkp.tile([128, N], MMD, name=f"{which}h{pr}", tag=f"{which}h{pr}")
            nc.vector.tensor_copy(out=t, in_=ps)
            lst[pr] = t

        def stats(pr, which):
            """1/sigma for one of q/k of pair pr via exp(-0.5*ln(...)) —
            same ACT table set as the attention Exp."""
            if sig_r[pr] is None:
                sig_r[pr] = {}
            for which, src, eps_t, lsc in (
                (("q", qh_sb[pr], epsq2, 1.0),) if which == "q"
                else (("k", kh_sb[pr], epsk2, 1.0 / D),)
            ):
                sq = work.tile([128, N], MMD, name=f"sq_{which}{pr}", tag="sq")
                nc.vector.tensor_mul(out=sq, in0=src, in1=src)
                ps2 = psG.tile([2, N], F32, name=f"ps_st{which}{pr}", tag="g")
                for nb in range(NB):
                    nc.tensor.matmul(
                        ps2[:, 512 * nb:512 * (nb + 1)],
                        lhsT=mm(seg_sb[:, pr, 2 * pr:2 * pr + 2]),
                        rhs=mm(sq[:, 512 * nb:512 * (nb + 1)]),
                        start=True, stop=True,
                    )
                ln_t = stp.tile([2, N], F32, name=f"ln{which}{pr}", tag="ln")
                nc.scalar.activation(
                    out=ln_t, in_=ps2, func=mybir.ActivationFunctionType.Ln,
                    bias=eps_t, scale=lsc,
                )
                inv = stp.tile([2, N], F32, name=f"inv{which}{pr}", tag="inv")
                nc.scalar.activation(
                    out=inv, in_=ln_t, func=mybir.ActivationFunctionType.Exp,
                    scale=-0.5,
                )
                sr = stp.tile([2, N], MMD, name=f"sigr{which}{pr}",
                              tag=f"sigr{which}", bufs=2)
                nc.vector.tensor_copy(out=sr, in_=inv)  # real cast: f32r rounds
                sig_r[pr][which] = sr

        def prescale(pr, which):
            tgt = qh_sb[pr] if which == "q" else kh_sb[pr]
            ps = psG.tile([128, N], F32, name=f"ps_b{which}{pr}", tag="g")
            for nb in range(NB):
                nc.tensor.matmul(
                    ps[:, 512 * nb:512 * (nb + 1)],
                    lhsT=mm(ind2[:, :]),
                    rhs=mm(sig_r[pr][which][:, 512 * nb:512 * (nb + 1)]),
                    start=True, stop=True,
                )
            nc.vector.tensor_mul(out=tgt, in0=tgt, in1=ps)

        def gen_v(mt):
            """V tile in bf16 with the ones column for softmax denominators."""
            ps = psG.tile([128, C], F32, name=f"ps_v{mt}", tag="g")
            for kc in range(CT):
                for vo, vn in ((0, 512), (512, 256)):
                    nc.tensor.matmul(
                        ps[:, vo:vo + vn],
                        lhsT=xtb_sb[kc][:, 128 * mt:128 * (mt + 1)],
                        rhs=wv_sb[kc][:, vo:vo + vn],
                        start=(kc == 0),
                        stop=(kc == CT - 1),
                    )
            t = persist.tile([128, H, D + 1], BF16, name=f"vsb{mt}", tag=f"v{mt}")
            nc.vector.memset(t, 1.0)
            nc.vector.tensor_copy(
                out=t[:, :, 0:D], in_=ps.rearrange("p (h d) -> p h d", h=H)
            )
            v_sb[mt] = t

        # ---- prefix: pair 0 (+ first V tile). gen-k borrows an (idle
        # until attention) psS slot so the k generation matmuls run while the
        # q stats/prescale chain serializes on the psG slot ----
        gen_qk(0, "q")
        gen_qk(0, "k", pstag="s")
        stats(0, "q")
        prescale(0, "q")
        stats(0, "k")
        prescale(0, "k")

        def epilogue(pr, nb, ot_ps):
            if ot_sb[pr] is None:
                # reuses qh[pr]'s slot — qh dies at this pair's last QK
                ot_sb[pr] = qkp.tile([128, N], BF16, name=f"ot{pr}", tag=f"qh{pr}")
            osb = []
            for j in range(2):
                t = epi.tile([D + 1, 512], F32, name=f"osb{pr}_{nb}_{j}", tag=f"osb{j}")
                nc.vector.tensor_copy(out=t, in_=ot_ps[j])
                osb.append(t)
            for j in range(2):
                h = 2 * pr + j
                rr = epi.tile([D + 1, 512], F32, name=f"rr{h}_{nb}", tag="rr")
                nc.vector.reciprocal_approx_fast(out=rr, in_=osb[j])
                dn_dram = dramp.tile([1, 512], F32, name=f"dnd{h}_{nb}", tag="dnd")
                nc.gpsimd.dma_start(out=dn_dram, in_=rr[D:D + 1, :])
                rbc = epi.tile([D, 512], F32, name=f"rbc{h}_{nb}", tag=f"rbc{j}")
                bcast_ap = bass.AP(
                    tensor=dn_dram.tensor, offset=dn_dram.offset,
                    ap=[[0, D]] + [list(a) for a in dn_dram.ap[1:]],
                )
                nc.gpsimd.dma_start(out=rbc, in_=bcast_ap)
                nsl = slice(512 * nb, 512 * (nb + 1))
                if j == 0:
                    nc.vector.tensor_mul(
                        out=ot_sb[pr][0:D, nsl], in0=osb[j][0:D, :], in1=rbc
                    )
                else:
                    tmpb = epi.tile([D, 512], BF16, name=f"tmpb{h}_{nb}", tag="tmpb")
                    nc.vector.tensor_mul(out=tmpb, in0=osb[j][0:D, :], in1=rbc)
                    nc.gpsimd.dma_start(out=ot_sb[pr][D:2 * D, nsl], in_=tmpb)

        pj_sb = [None] * NT

        def proj_partial(nt):
            """Projection contribution of c-tiles 0..4 for token tile nt —
            runs as pair-5 filler (ot[0..4] are final by then); only the
            kc=5 term + an add remain after the last epilogue."""
            ps = psG.tile([128, C], F32, name=f"ps_pjp{nt}", tag="g")
            for kc in range(CT - 1):
                for vo, vn in ((0, 512), (512, 256)):
                    nc.tensor.matmul(
                        ps[:, vo:vo + vn],
                        lhsT=ot_sb[kc][:, 128 * nt:128 * (nt + 1)],
                        rhs=wp_sb[kc][:, vo:vo + vn],
                        start=(kc == 0),
                        stop=(kc == CT - 2),
                    )
            t = persist.tile([128, C], F32, name=f"pjp{nt}", tag=f"pj{nt}")
            nc.vector.tensor_copy(out=t, in_=ps)
            pj_sb[nt] = t

        # Flat software-pipelined attention stream over all 96 (pr, nb, mt)
        # iterations with QK lookahead-1: per iteration emit QK(g+1), then
        # Exp(g), then this iteration's filler chunks (next-pair generation /
        # V tiles on the dedicated psG slot), then PV(g). The lookahead keeps
        # the ACT queue fed across iteration and pair boundaries; the filler
        # keeps the PE stream dense (HAM stays at full clock).
        iters = [(pr, nb, mt) for pr in range(NP) for nb in range(NB)
                 for mt in range(NT)]
        G = len(iters)

        sched = {}
        for mt in range(NT - 1):       # V[0] in prefix; V[mt+1] ready by PV(mt+1)
            sched[mt] = [lambda m=mt + 1: gen_v(m)]
        for nt in range(NT):
            sched.setdefault(5 * 16 + 1 + nt, []).append(
                lambda n=nt: proj_partial(n))
        for pr in range(NP - 1):
            base = pr * 16 + 8
            chunks = [
                lambda p=pr + 1: gen_qk(p, "q"),
                lambda p=pr + 1: gen_qk(p, "k"),
                lambda p=pr + 1: stats(p, "q"),
                lambda p=pr + 1: stats(p, "k"),
                lambda p=pr + 1: prescale(p, "q"),
                lambda p=pr + 1: prescale(p, "k"),
            ]
            for off, ch in enumerate(chunks):
                sched.setdefault(base + off, []).append(ch)

        gen_v(0)
        wp_sb = load_w(wp, "wp", vp_pool, "vp", BF16, nc.sync)
        s_tiles = {}
        pt_tiles = {}
        ot_cur = [None, None]

        def emit_qk(g):
            pr, nb, mt = iters[g]
            s_ps = psS.tile([128, N], F32, name=f"ps_s{g}", tag="s")
            for j in range(2):
                nc.tensor.matmul(
                    s_ps[:, 512 * j:512 * (j + 1)],
                    lhsT=mm(kh_sb[pr][64 * j:64 * (j + 1), 128 * mt:128 * (mt + 1)]),
                    rhs=mm(qh_sb[pr][64 * j:64 * (j + 1), 512 * nb:512 * (nb + 1)]),
                    start=True, stop=True,
                )
            s_tiles[g] = s_ps

        emit_qk(0)
        for g in range(G):
            pr, nb, mt = iters[g]
            if g + 1 < G:
                emit_qk(g + 1)
            pt = ptp.tile([128, N], BF16, name=f"pt{g}", tag="pt")
            nc.scalar.activation(
                out=pt, in_=s_tiles.pop(g),
                func=mybir.ActivationFunctionType.Exp,
            )
            for ch in sched.get(g, ()):
                ch()
            if mt == 0:
                ot_cur = [
                    psOT.tile([D + 1, 512], F32, name=f"ps_ot{pr}_{nb}_{j}",
                              tag=f"ot{j}")
                    for j in range(2)
                ]
            for j in range(2):
                nc.tensor.matmul(
                    ot_cur[j][:, :],
                    lhsT=v_sb[mt][:, 2 * pr + j, :],
                    rhs=pt[:, 512 * j:512 * (j + 1)],
                    start=(mt == 0),
                    stop=(mt == NT - 1),
                )
            if mt == NT - 1:
                epilogue(pr, nb, ot_cur)

        # ---- projection finish: kc=5 term + add of the staged partials ----
        for nt in range(NT):
            ps = psS.tile([128, C], F32, name=f"ps_pj{nt}", tag="s")
            for vo, vn in ((0, 512), (512, 256)):
                nc.tensor.matmul(
                    ps[:, vo:vo + vn],
                    lhsT=ot_sb[CT - 1][:, 128 * nt:128 * (nt + 1)],
                    rhs=wp_sb[CT - 1][:, vo:vo + vn],
                    start=True, stop=True,
                )
            osb = work.tile([128, C], F32, name=f"osb{nt}", tag="sq")
            nc.vector.tensor_add(out=osb, in0=pj_sb[nt], in1=ps)
            eng = nc.sync if nt % 2 == 0 else nc.scalar
            eng.dma_start(out=out_ext[128 * nt:128 * (nt + 1), :], in_=osb)

    if compile_module:
        nc.compile()
    return nc


def prep_inputs(x, qkv_weight, proj_weight):
    import ml_dtypes
    x = np.asarray(x, dtype=np.float32)
    qkv_weight = np.asarray(qkv_weight, dtype=np.float32)
    proj_weight = np.asarray(proj_weight, dtype=np.float32)

    Wq = qkv_weight[0:C]
    Wk = qkv_weight[C:2 * C]
    Wv = qkv_weight[2 * C:3 * C]

    def center(W):
        Wc = W.reshape(H, D, C)
        Wc = Wc - Wc.mean(axis=1, keepdims=True)
        return Wc.reshape(C, C)

    wqcT = np.ascontiguousarray(center(Wq).T).astype(ml_dtypes.bfloat16)
    wkcT = np.ascontiguousarray(center(Wk).T).astype(ml_dtypes.bfloat16)
    wvT = np.ascontiguousarray(Wv.T).astype(ml_dtypes.bfloat16)
    wpT = np.ascontiguousarray(proj_weight.T).astype(ml_dtypes.bfloat16)

    seg = np.zeros((128, CT, H), ml_dtypes.bfloat16)
    for r in range(CT):
        for j in range(2):
            seg[64 * j:64 * (j + 1), r, 2 * r + j] = 1.0
    ind2 = np.zeros((2, 128), ml_dtypes.bfloat16)
    ind2[0, 0:64] = 1.0
    ind2[1, 64:128] = 1.0

    in_maps = []
    for b in range(x.shape[0]):
        xt = np.ascontiguousarray(x[b].T)
        in_maps.append(dict(
            xTb=xt.astype(ml_dtypes.bfloat16),
            wqcT=wqcT, wkcT=wkcT, wvT=wvT, wpT=wpT,
            seg=seg, ind2=ind2,
        ))
    return in_maps


_CACHE = {}


def kernel(x, qkv_weight, proj_weight):
    if "nc" not in _CACHE:
        _CACHE["nc"] = build()
    nc = _CACHE["nc"]
    in_maps = prep_inputs(x, qkv_weight, proj_weight)
    from concourse.bass_utils import run_bass_kernel_spmd
    res = run_bass_kernel_spmd(nc, in_maps, core_ids=list(range(len(in_maps))))
    out = np.stack([res.results[i]["out"] for i in range(len(in_maps))], axis=0)
    return out.astype(np.float32)


# revision 23
# speedup vs baseline: 1.0104x; 1.0104x over previous
"""Trainium2 Bass kernel for nn_AttentionCompiled (dense transformer attention).

B=8, N=1024, C=768, H=12 heads, D=64. Per-head LayerNorm on q/k, softmax
attention, output projection. Pure data parallelism: one batch element per
NeuronCore, weights replicated, no collectives.

Math folding:
 - LN centering folded into Wq/Wk (CPU-side).
 - 1/sigma_q (with the 1/sqrt(D) scale) folded into q and 1/sigma_k into k via
   PE-broadcast matmul + DVE multiply, so Exp runs with scale=1 and both
   512-wide query blocks batch into one FD=1024 ACTIVATE.
 - 1/sigma = exp(-0.5*ln(scale*sumsq + eps)): ln and exp share ONE ACT table
   set (natural_log_exp_and_others), so per-pair stats interleave with
   attention exps with zero table reloads. build() pins that choice by
   emptying the single-function exp/ln sets in the table map (index-preserving
   — act_func_set_id indexes act_info.json).
 - Softmax denominators from an appended ones-column on V (row 64 of O^T).
 - |scores| <= 8 (Cauchy-Schwarz on LN'd vectors): exp needs no max-subtract.

Precision split: q/k generation and QK^T run in f32r (score exponents are
error-sensitive); the V path (x_bf16 @ Wv), P (exp output), O^T and the
projection run in bf16 with fp32 PSUM accumulation (gate is 2e-2).

Perf structure (v3):
 - Per-pair software pipeline: pair p's ACT-paced attention stream carries
   pair p+1's generation/stats/prescale matmuls (and, for pair 0, the whole
   V generation) as PE filler, so the PE instruction stream stays dense —
   idle PE windows re-engage the HAM clock gate (K=4/8, half clock), which
   is what capped v1/v2.
 - Loop order pr -> nb -> mt; score tile [128,1024] holds both heads
   (row-group concurrent QK); one Exp ACTIVATE per tile (FD=1024).
 - PSUM: 2x score tiles (4 banks) + 2x2 O^T accumulators [65,512] (4 banks,
   double-buffered so the next pair's PV never waits the epilogue copies).
 - Epilogue per (pr, nb): copy accumulators out, reciprocal_approx_fast
   (full-tile: the custom DVE op mishandles base_partition != 0), DMA
   row-broadcast via DRAM bounce, DVE normalize into bf16 O^T.
"""

import sys
import numpy as np
from contextlib import ExitStack

if "/opt/trn_rl_repo" not in sys.path:
    sys.path.insert(0, "/opt/trn_rl_repo")

import concourse.bass as bass
import concourse.bacc as bacc
import concourse.tile as tile
from concourse import mybir

F32 = mybir.dt.float32
F32R = mybir.dt.float32r
BF16 = mybir.dt.bfloat16

N = 1024
C = 768
H = 12
D = 64
NT = N // 128
CT = C // 128
NB = N // 512
NP = H // 2
EPS = 1e-5

USE_F32R = True


def _filtered_act_tables(arch):
    import concourse.hw_specs as hw_specs
    tabs = dict(hw_specs.get_activation_tables(arch))
    # empty them (never match) rather than delete: act_func_set_id is the
    # INDEX into this ordered dict and must stay aligned with act_info.json
    for k in ("exp_and_others", "natural_log", "exp_and_friends"):
        tabs[k] = set()
    return tabs


def build(use_f32r: bool = USE_F32R, compile_module: bool = True) -> bass.Bass:
    bacc.get_activation_tables = _filtered_act_tables
    nc = bacc.Bacc()

    xTb = nc.declare_dram_parameter("xTb", [C, N], BF16, isOutput=False)
    wq = nc.declare_dram_parameter("wqcT", [C, C], BF16, isOutput=False)
    wk = nc.declare_dram_parameter("wkcT", [C, C], BF16, isOutput=False)
    wv = nc.declare_dram_parameter("wvT", [C, C], BF16, isOutput=False)
    wp = nc.declare_dram_parameter("wpT", [C, C], BF16, isOutput=False)
    seg = nc.declare_dram_parameter("seg", [128, CT, H], BF16, isOutput=False)
    ind2d = nc.declare_dram_parameter("ind2", [2, 128], BF16, isOutput=False)
    out_ext = nc.declare_dram_parameter("out", [N, C], F32, isOutput=True)

    MMD = BF16

    def mm(ap):
        return ap

    with tile.TileContext(nc) as tc, ExitStack() as ctx:
        xo_pool = ctx.enter_context(tc.tile_pool(name="xo", bufs=6))
        persist = ctx.enter_context(tc.tile_pool(name="persist", bufs=1))
        vp_pool = ctx.enter_context(tc.tile_pool(name="vp", bufs=6))
        work = ctx.enter_context(tc.tile_pool(name="work", bufs=2))
        ptp = ctx.enter_context(tc.tile_pool(name="ptp", bufs=3))
        epi = ctx.enter_context(tc.tile_pool(name="epi", bufs=1))
        stp = ctx.enter_context(tc.tile_pool(name="stp", bufs=1))
        qkp = ctx.enter_context(tc.tile_pool(name="qkp", bufs=1))
        rows = ctx.enter_context(tc.tile_pool(name="rows", bufs=1))
        dramp = ctx.enter_context(tc.tile_pool(name="dramp", bufs=2, space="DRAM"))
        psS = ctx.enter_context(tc.tile_pool(name="psS", bufs=2, space="PSUM"))
        psG = ctx.enter_context(tc.tile_pool(name="psG", bufs=1, space="PSUM"))
        psOT = ctx.enter_context(tc.tile_pool(name="psOT", bufs=1, space="PSUM"))

        # ---- loads: spread across engine DMA queues so the prefix isn't
        # serialized on one queue, and interleave (wq[kc], xt[kc]) so the
        # first gen matmuls can start after the first c-tile lands ----
        def load_w(dram, nm, pool, tag, dt, eng, per_tile_tags=False):
            tiles = []
            for r in range(CT):
                t = pool.tile(
                    [128, C], dt, name=f"{nm}{r}",
                    tag=(f"{tag}{r}" if per_tile_tags else tag),
                )
                eng.dma_start(out=t, in_=mm(dram[128 * r:128 * (r + 1), :]))
                tiles.append(t)
            return tiles

        seg_sb = rows.tile([128, CT, H], MMD, name="seg", tag="seg")
        nc.sync.dma_start(out=seg_sb, in_=mm(seg[:, :, :]))
        ind2 = rows.tile([2, 128], MMD, name="ind2", tag="ind2")
        nc.sync.dma_start(out=ind2, in_=mm(ind2d[:, :]))

        xtb_sb = []
        wq_sb = load_w(wq, "wq", persist, "wq", BF16, nc.scalar, per_tile_tags=True)
        for r in range(CT):
            t = xo_pool.tile([128, N], BF16, name=f"xtb{r}", tag="xb")
            nc.sync.dma_start(out=t, in_=xTb[128 * r:128 * (r + 1), :])
            xtb_sb.append(t)
        wk_sb = load_w(wk, "wk", persist, "wk", BF16, nc.gpsimd, per_tile_tags=True)
        wv_sb = load_w(wv, "wv", vp_pool, "vp", BF16, nc.gpsimd)
        xt_sb = xtb_sb

        epsq2 = rows.tile([2, 1], F32, name="epsq2", tag="epsq2")
        nc.vector.memset(epsq2, float(D) * EPS)
        epsk2 = rows.tile([2, 1], F32, name="epsk2", tag="epsk2")
        nc.vector.memset(epsk2, EPS)

        qh_sb = [None] * NP
        kh_sb = [None] * NP
        sig_r = [None] * NP
        v_sb = [None] * NT
        ot_sb = [None] * CT

        def gen_qk(pr, which, pstag="g"):
            wt = wq_sb if which == "q" else wk_sb
            lst = qh_sb if which == "q" else kh_sb
            pool = psG if pstag == "g" else psS
            ps = pool.tile([128, N], F32, name=f"ps_{which}{pr}", tag=pstag)
            for kc in range(CT):
                for nb in range(NB):
                    nc.tensor.matmul(
                        ps[:, 512 * nb:512 * (nb + 1)],
                        lhsT=mm(wt[kc][:, 128 * pr:128 * (pr + 1)]),
                        rhs=mm(xt_sb[kc][:, 512 * nb:512 * (nb + 1)]),
                        start=(kc == 0),
                        stop=(kc == CT - 1),
                    )
            t = qkp.tile([128, N], MMD, name=f"{which}h{pr}", tag=f"{which}h{pr}")
            nc.vector.tensor_copy(out=t, in_=ps)
            lst[pr] = t

        def stats(pr, which):
            """1/sigma for one of q/k of pair pr via exp(-0.5*ln(...)) —
            same ACT table set as the attention Exp."""
            if sig_r[pr] is None:
                sig_r[pr] = {}
            for which, src, eps_t, lsc in (
                (("q", qh_sb[pr], epsq2, 1.0),) if which == "q"
                else (("k", kh_sb[pr], epsk2, 1.0 / D),)
            ):
                sq = work.tile([128, N], MMD, name=f"sq_{which}{pr}", tag="sq")
                nc.gpsimd.tensor_mul(out=sq, in0=src, in1=src)
                ps2 = psG.tile([2, N], F32, name=f"ps_st{which}{pr}", tag="g")
                for nb in range(NB):
                    nc.tensor.matmul(
                        ps2[:, 512 * nb:512 * (nb + 1)],
                        lhsT=mm(seg_sb[:, pr, 2 * pr:2 * pr + 2]),
                        rhs=mm(sq[:, 512 * nb:512 * (nb + 1)]),
                        start=True, stop=True,
                    )
                ln_t = stp.tile([2, N], F32, name=f"ln{which}{pr}", tag="ln")
                nc.scalar.activation(
                    out=ln_t, in_=ps2, func=mybir.ActivationFunctionType.Ln,
                    bias=eps_t, scale=lsc,
                )
                inv = stp.tile([2, N], F32, name=f"inv{which}{pr}", tag="inv")
                nc.scalar.activation(
                    out=inv, in_=ln_t, func=mybir.ActivationFunctionType.Exp,
                    scale=-0.5,
                )
                sr = stp.tile([2, N], MMD, name=f"sigr{which}{pr}",
                              tag=f"sigr{which}", bufs=2)
                nc.vector.tensor_copy(out=sr, in_=inv)  # real cast: f32r rounds
                sig_r[pr][which] = sr

        def prescale(pr, which):
            tgt = qh_sb[pr] if which == "q" else kh_sb[pr]
            ps = psG.tile([128, N], F32, name=f"ps_b{which}{pr}", tag="g")
            for nb in range(NB):
                nc.tensor.matmul(
                    ps[:, 512 * nb:512 * (nb + 1)],
                    lhsT=mm(ind2[:, :]),
                    rhs=mm(sig_r[pr][which][:, 512 * nb:512 * (nb + 1)]),
                    start=True, stop=True,
                )
            nc.vector.tensor_mul(out=tgt, in0=tgt, in1=ps)

        def gen_v(mt):
            """V tile in bf16 with the ones column for softmax denominators."""
            ps = psG.tile([128, C], F32, name=f"ps_v{mt}", tag="g")
            for kc in range(CT):
                for vo, vn in ((0, 512), (512, 256)):
                    nc.tensor.matmul(
                        ps[:, vo:vo + vn],
                        lhsT=xtb_sb[kc][:, 128 * mt:128 * (mt + 1)],
                        rhs=wv_sb[kc][:, vo:vo + vn],
                        start=(kc == 0),
                        stop=(kc == CT - 1),
                    )
            t = persist.tile([128, H, D + 1], BF16, name=f"vsb{mt}", tag=f"v{mt}")
            nc.vector.memset(t, 1.0)
            nc.vector.tensor_copy(
                out=t[:, :, 0:D], in_=ps.rearrange("p (h d) -> p h d", h=H)
            )
            v_sb[mt] = t

        # ---- prefix: pair 0 (+ first V tile). gen-k borrows an (idle
        # until attention) psS slot so the k generation matmuls run while the
        # q stats/prescale chain serializes on the psG slot ----
        gen_qk(0, "q")
        gen_qk(0, "k", pstag="s")
        stats(0, "q")
        prescale(0, "q")
        stats(0, "k")
        prescale(0, "k")

        def epilogue(pr, nb, ot_ps):
            if ot_sb[pr] is None:
                # reuses qh[pr]'s slot — qh dies at this pair's last QK
                ot_sb[pr] = qkp.tile([128, N], BF16, name=f"ot{pr}", tag=f"qh{pr}")
            osb = []
            for j in range(2):
                t = epi.tile([D + 1, 512], F32, name=f"osb{pr}_{nb}_{j}", tag=f"osb{j}")
                nc.vector.tensor_copy(out=t, in_=ot_ps[j])
                osb.append(t)
            for j in range(2):
                h = 2 * pr + j
                rr = epi.tile([D + 1, 512], F32, name=f"rr{h}_{nb}", tag="rr")
                nc.vector.reciprocal_approx_fast(out=rr, in_=osb[j])
                dn_dram = dramp.tile([1, 512], F32, name=f"dnd{h}_{nb}", tag="dnd")
                nc.gpsimd.dma_start(out=dn_dram, in_=rr[D:D + 1, :])
                rbc = epi.tile([D, 512], F32, name=f"rbc{h}_{nb}", tag=f"rbc{j}")
                bcast_ap = bass.AP(
                    tensor=dn_dram.tensor, offset=dn_dram.offset,
                    ap=[[0, D]] + [list(a) for a in dn_dram.ap[1:]],
                )
                nc.gpsimd.dma_start(out=rbc, in_=bcast_ap)
                nsl = slice(512 * nb, 512 * (nb + 1))
                if j == 0:
                    nc.gpsimd.tensor_mul(
                        out=ot_sb[pr][0:D, nsl], in0=osb[j][0:D, :], in1=rbc
                    )
                else:
                    tmpb = epi.tile([D, 512], BF16, name=f"tmpb{h}_{nb}", tag="tmpb")
                    nc.gpsimd.tensor_mul(out=tmpb, in0=osb[j][0:D, :], in1=rbc)
                    nc.gpsimd.dma_start(out=ot_sb[pr][D:2 * D, nsl], in_=tmpb)

        pj_sb = [None] * NT

        _pjps = {}

        def proj_partial(nt, half):
            """Projection contribution of c-tiles 0..4 for token tile nt —
            runs as pair-5 filler (ot[0..4] are final by then); only the
            kc=5 term + an add remain after the last epilogue. Split in two
            filler-sized halves."""
            if half == 0:
                _pjps[nt] = psG.tile([128, C], F32, name=f"ps_pjp{nt}", tag="g")
            ps = _pjps[nt]
            kcs = (0, 1, 2) if half == 0 else (3, 4)
            for kc in kcs:
                for vo, vn in ((0, 512), (512, 256)):
                    nc.tensor.matmul(
                        ps[:, vo:vo + vn],
                        lhsT=ot_sb[kc][:, 128 * nt:128 * (nt + 1)],
                        rhs=wp_sb[kc][:, vo:vo + vn],
                        start=(kc == 0),
                        stop=(kc == CT - 2),
                    )
            if half == 1:
                t = persist.tile([128, C], F32, name=f"pjp{nt}", tag=f"pj{nt}")
                nc.vector.tensor_copy(out=t, in_=ps)
                pj_sb[nt] = t

        # Flat software-pipelined attention stream over all 96 (pr, nb, mt)
        # iterations with QK lookahead-1: per iteration emit QK(g+1), then
        # Exp(g), then this iteration's filler chunks (next-pair generation /
        # V tiles on the dedicated psG slot), then PV(g). The lookahead keeps
        # the ACT queue fed across iteration and pair boundaries; the filler
        # keeps the PE stream dense (HAM stays at full clock).
        iters = [(pr, nb, mt) for pr in range(NP) for nb in range(NB)
                 for mt in range(NT)]
        G = len(iters)

        sched = {}
        for mt in range(NT - 1):       # V[0] in prefix; V[mt+1] ready by PV(mt+1)
            sched[mt] = [lambda m=mt + 1: gen_v(m)]
        for nt in range(NT):
            for half in (0, 1):
                g = 5 * 16 + (2 * nt + half) % 16
                sched.setdefault(g, []).append(
                    lambda n=nt, hf=half: proj_partial(n, hf))
        for pr in range(NP - 1):
            base = pr * 16 + 8
            chunks = [
                lambda p=pr + 1: gen_qk(p, "q"),
                lambda p=pr + 1: gen_qk(p, "k"),
                lambda p=pr + 1: stats(p, "q"),
                lambda p=pr + 1: stats(p, "k"),
                lambda p=pr + 1: prescale(p, "q"),
                lambda p=pr + 1: prescale(p, "k"),
            ]
            for off, ch in enumerate(chunks):
                sched.setdefault(base + off, []).append(ch)

        gen_v(0)
        wp_sb = load_w(wp, "wp", vp_pool, "vp", BF16, nc.sync)
        s_tiles = {}
        pt_tiles = {}
        ot_cur = [None, None]

        def emit_qk(g):
            pr, nb, mt = iters[g]
            s_ps = psS.tile([128, N], F32, name=f"ps_s{g}", tag="s")
            for j in range(2):
                nc.tensor.matmul(
                    s_ps[:, 512 * j:512 * (j + 1)],
                    lhsT=mm(kh_sb[pr][64 * j:64 * (j + 1), 128 * mt:128 * (mt + 1)]),
                    rhs=mm(qh_sb[pr][64 * j:64 * (j + 1), 512 * nb:512 * (nb + 1)]),
                    start=True, stop=True,
                )
            s_tiles[g] = s_ps

        emit_qk(0)
        for g in range(G):
            pr, nb, mt = iters[g]
            if g + 1 < G:
                emit_qk(g + 1)
            pt = ptp.tile([128, N], BF16, name=f"pt{g}", tag="pt")
            nc.scalar.activation(
                out=pt, in_=s_tiles.pop(g),
                func=mybir.ActivationFunctionType.Exp,
            )
            for ch in sched.get(g, ()):
                ch()
            if mt == 0:
                ot_cur = [
                    psOT.tile([D + 1, 512], F32, name=f"ps_ot{pr}_{nb}_{j}",
                              tag=f"ot{j}")
                    for j in range(2)
                ]
            for j in range(2):
                nc.tensor.matmul(
                    ot_cur[j][:, :],
                    lhsT=v_sb[mt][:, 2 * pr + j, :],
                    rhs=pt[:, 512 * j:512 * (j + 1)],
                    start=(mt == 0),
                    stop=(mt == NT - 1),
                )
            if mt == NT - 1:
                epilogue(pr, nb, ot_cur)

        # ---- projection finish: kc=5 term + add of the staged partials ----
        for nt in range(NT):
            ps = psS.tile([128, C], F32, name=f"ps_pj{nt}", tag="s")
            for vo, vn in ((0, 512), (512, 256)):
                nc.tensor.matmul(
                    ps[:, vo:vo + vn],
                    lhsT=ot_sb[CT - 1][:, 128 * nt:128 * (nt + 1)],
                    rhs=wp_sb[CT - 1][:, vo:vo + vn],
                    start=True, stop=True,
                )
            osb = work.tile([128, C], F32, name=f"osb{nt}", tag="sq")
            nc.vector.tensor_add(out=osb, in0=pj_sb[nt], in1=ps)
            eng = nc.sync if nt % 2 == 0 else nc.scalar
            eng.dma_start(out=out_ext[128 * nt:128 * (nt + 1), :], in_=osb)

    if compile_module:
        nc.compile()
    return nc


def prep_inputs(x, qkv_weight, proj_weight):
    import ml_dtypes
    x = np.asarray(x, dtype=np.float32)
    qkv_weight = np.asarray(qkv_weight, dtype=np.float32)
    proj_weight = np.asarray(proj_weight, dtype=np.float32)

    Wq = qkv_weight[0:C]
    Wk = qkv_weight[C:2 * C]
    Wv = qkv_weight[2 * C:3 * C]

    def center(W):
        Wc = W.reshape(H, D, C)
        Wc = Wc - Wc.mean(axis=1, keepdims=True)
        return Wc.reshape(C, C)

    wqcT = np.ascontiguousarray(center(Wq).T).astype(ml_dtypes.bfloat16)
    wkcT = np.ascontiguousarray(center(Wk).T).astype(ml_dtypes.bfloat16)
    wvT = np.ascontiguousarray(Wv.T).astype(ml_dtypes.bfloat16)
    wpT = np.ascontiguousarray(proj_weight.T).astype(ml_dtypes.bfloat16)

    seg = np.zeros((128, CT, H), ml_dtypes.bfloat16)
    for r in range(CT):
        for j in range(2):
            seg[64 * j:64 * (j + 1), r, 2 * r + j] = 1.0
    ind2 = np.zeros((2, 128), ml_dtypes.bfloat16)
    ind2[0, 0:64] = 1.0
    ind2[1, 64:128] = 1.0

    in_maps = []
    for b in range(x.shape[0]):
        xt = np.ascontiguousarray(x[b].T)
        in_maps.append(dict(
            xTb=xt.astype(ml_dtypes.bfloat16),
            wqcT=wqcT, wkcT=wkcT, wvT=wvT, wpT=wpT,
            seg=seg, ind2=ind2,
        ))
    return in_maps


_CACHE = {}


def kernel(x, qkv_weight, proj_weight):
    if "nc" not in _CACHE:
        _CACHE["nc"] = build()
    nc = _CACHE["nc"]
    in_maps = prep_inputs(x, qkv_weight, proj_weight)
    from concourse.bass_utils import run_bass_kernel_spmd
    res = run_bass_kernel_spmd(nc, in_maps, core_ids=list(range(len(in_maps))))
    out = np.stack([res.results[i]["out"] for i in range(len(in_maps))], axis=0)
    return out.astype(np.float32)


# revision 24
# speedup vs baseline: 1.0652x; 1.0542x over previous
"""Trainium2 Bass kernel for nn_AttentionCompiled (dense transformer attention).

B=8, N=1024, C=768, H=12 heads, D=64. Per-head LayerNorm on q/k, softmax
attention, output projection. Pure data parallelism: one batch element per
NeuronCore, weights replicated, no collectives.

Math folding:
 - LN centering folded into Wq/Wk (CPU-side).
 - 1/sigma_q (with the 1/sqrt(D) scale) folded into q and 1/sigma_k into k via
   PE-broadcast matmul + DVE multiply, so Exp runs with scale=1 and both
   512-wide query blocks batch into one FD=1024 ACTIVATE.
 - 1/sigma = exp(-0.5*ln(scale*sumsq + eps)): ln and exp share ONE ACT table
   set (natural_log_exp_and_others), so per-pair stats interleave with
   attention exps with zero table reloads. build() pins that choice by
   emptying the single-function exp/ln sets in the table map (index-preserving
   — act_func_set_id indexes act_info.json).
 - Softmax denominators from an appended ones-column on V (row 64 of O^T).
 - |scores| <= 8 (Cauchy-Schwarz on LN'd vectors): exp needs no max-subtract.

Precision split: q/k generation and QK^T run in f32r (score exponents are
error-sensitive); the V path (x_bf16 @ Wv), P (exp output), O^T and the
projection run in bf16 with fp32 PSUM accumulation (gate is 2e-2).

Perf structure (v3):
 - Per-pair software pipeline: pair p's ACT-paced attention stream carries
   pair p+1's generation/stats/prescale matmuls (and, for pair 0, the whole
   V generation) as PE filler, so the PE instruction stream stays dense —
   idle PE windows re-engage the HAM clock gate (K=4/8, half clock), which
   is what capped v1/v2.
 - Loop order pr -> nb -> mt; score tile [128,1024] holds both heads
   (row-group concurrent QK); one Exp ACTIVATE per tile (FD=1024).
 - PSUM: 2x score tiles (4 banks) + 2x2 O^T accumulators [65,512] (4 banks,
   double-buffered so the next pair's PV never waits the epilogue copies).
 - Epilogue per (pr, nb): copy accumulators out, reciprocal_approx_fast
   (full-tile: the custom DVE op mishandles base_partition != 0), DMA
   row-broadcast via DRAM bounce, DVE normalize into bf16 O^T.
"""

import sys
import numpy as np
from contextlib import ExitStack

if "/opt/trn_rl_repo" not in sys.path:
    sys.path.insert(0, "/opt/trn_rl_repo")

import concourse.bass as bass
import concourse.bacc as bacc
import concourse.tile as tile
from concourse import mybir

F32 = mybir.dt.float32
F32R = mybir.dt.float32r
BF16 = mybir.dt.bfloat16

N = 1024
C = 768
H = 12
D = 64
NT = N // 128
CT = C // 128
NB = N // 512
NP = H // 2
EPS = 1e-5

USE_F32R = True


def _filtered_act_tables(arch):
    import concourse.hw_specs as hw_specs
    tabs = dict(hw_specs.get_activation_tables(arch))
    # empty them (never match) rather than delete: act_func_set_id is the
    # INDEX into this ordered dict and must stay aligned with act_info.json
    for k in ("exp_and_others", "natural_log", "exp_and_friends"):
        tabs[k] = set()
    return tabs


def build(use_f32r: bool = USE_F32R, compile_module: bool = True) -> bass.Bass:
    bacc.get_activation_tables = _filtered_act_tables
    nc = bacc.Bacc()

    xTb = nc.declare_dram_parameter("xTb", [C, N], BF16, isOutput=False)
    wq = nc.declare_dram_parameter("wqcT", [C, C], BF16, isOutput=False)
    wk = nc.declare_dram_parameter("wkcT", [C, C], BF16, isOutput=False)
    wv = nc.declare_dram_parameter("wvT", [C, C], BF16, isOutput=False)
    wp = nc.declare_dram_parameter("wpT", [C, C], BF16, isOutput=False)
    seg = nc.declare_dram_parameter("seg", [128, CT, H], BF16, isOutput=False)
    ind2d = nc.declare_dram_parameter("ind2", [2, 128], BF16, isOutput=False)
    out_ext = nc.declare_dram_parameter("out", [N, C], F32, isOutput=True)

    MMD = BF16

    def mm(ap):
        return ap

    with tile.TileContext(nc) as tc, ExitStack() as ctx:
        xo_pool = ctx.enter_context(tc.tile_pool(name="xo", bufs=6))
        persist = ctx.enter_context(tc.tile_pool(name="persist", bufs=1))
        vp_pool = ctx.enter_context(tc.tile_pool(name="vp", bufs=6))
        work = ctx.enter_context(tc.tile_pool(name="work", bufs=2))
        ptp = ctx.enter_context(tc.tile_pool(name="ptp", bufs=3))
        epi = ctx.enter_context(tc.tile_pool(name="epi", bufs=1))
        stp = ctx.enter_context(tc.tile_pool(name="stp", bufs=1))
        qkp = ctx.enter_context(tc.tile_pool(name="qkp", bufs=1))
        rows = ctx.enter_context(tc.tile_pool(name="rows", bufs=1))
        dramp = ctx.enter_context(tc.tile_pool(name="dramp", bufs=2, space="DRAM"))
        psS = ctx.enter_context(tc.tile_pool(name="psS", bufs=2, space="PSUM"))
        psG = ctx.enter_context(tc.tile_pool(name="psG", bufs=1, space="PSUM"))
        psOT = ctx.enter_context(tc.tile_pool(name="psOT", bufs=1, space="PSUM"))

        # ---- loads: spread across engine DMA queues so the prefix isn't
        # serialized on one queue, and interleave (wq[kc], xt[kc]) so the
        # first gen matmuls can start after the first c-tile lands ----
        def load_w(dram, nm, pool, tag, dt, eng, per_tile_tags=False):
            tiles = []
            for r in range(CT):
                t = pool.tile(
                    [128, C], dt, name=f"{nm}{r}",
                    tag=(f"{tag}{r}" if per_tile_tags else tag),
                )
                eng.dma_start(out=t, in_=mm(dram[128 * r:128 * (r + 1), :]))
                tiles.append(t)
            return tiles

        seg_sb = rows.tile([128, CT, H], MMD, name="seg", tag="seg")
        nc.sync.dma_start(out=seg_sb, in_=mm(seg[:, :, :]))
        ind2 = rows.tile([2, 128], MMD, name="ind2", tag="ind2")
        nc.sync.dma_start(out=ind2, in_=mm(ind2d[:, :]))

        xtb_sb = []
        wq_sb = load_w(wq, "wq", persist, "wq", BF16, nc.scalar, per_tile_tags=True)
        for r in range(CT):
            t = xo_pool.tile([128, N], BF16, name=f"xtb{r}", tag="xb")
            nc.sync.dma_start(out=t, in_=xTb[128 * r:128 * (r + 1), :])
            xtb_sb.append(t)
        wk_sb = load_w(wk, "wk", persist, "wk", BF16, nc.gpsimd, per_tile_tags=True)
        wv_sb = load_w(wv, "wv", vp_pool, "vp", BF16, nc.gpsimd)
        xt_sb = xtb_sb

        epsq2 = rows.tile([2, 1], F32, name="epsq2", tag="epsq2")
        nc.vector.memset(epsq2, float(D) * EPS)
        epsk2 = rows.tile([2, 1], F32, name="epsk2", tag="epsk2")
        nc.vector.memset(epsk2, EPS)

        qh_sb = [None] * NP
        kh_sb = [None] * NP
        sig_r = [None] * NP
        v_sb = [None] * NT
        ot_sb = [None] * CT

        def gen_qk(pr, which, pstag="g"):
            wt = wq_sb if which == "q" else wk_sb
            lst = qh_sb if which == "q" else kh_sb
            pool = psG if pstag == "g" else psS
            ps = pool.tile([128, N], F32, name=f"ps_{which}{pr}", tag=pstag)
            for kc in range(CT):
                for nb in range(NB):
                    nc.tensor.matmul(
                        ps[:, 512 * nb:512 * (nb + 1)],
                        lhsT=mm(wt[kc][:, 128 * pr:128 * (pr + 1)]),
                        rhs=mm(xt_sb[kc][:, 512 * nb:512 * (nb + 1)]),
                        start=(kc == 0),
                        stop=(kc == CT - 1),
                    )
            t = qkp.tile([128, N], MMD, name=f"{which}h{pr}", tag=f"{which}h{pr}")
            nc.vector.tensor_copy(out=t, in_=ps)
            lst[pr] = t

        def stats(pr, which):
            """1/sigma for one of q/k of pair pr via exp(-0.5*ln(...)) —
            same ACT table set as the attention Exp."""
            if sig_r[pr] is None:
                sig_r[pr] = {}
            for which, src, eps_t, lsc in (
                (("q", qh_sb[pr], epsq2, 1.0),) if which == "q"
                else (("k", kh_sb[pr], epsk2, 1.0 / D),)
            ):
                sq = work.tile([128, N], MMD, name=f"sq_{which}{pr}", tag="sq")
                nc.vector.tensor_mul(out=sq, in0=src, in1=src)
                ps2 = psG.tile([2, N], F32, name=f"ps_st{which}{pr}", tag="g")
                for nb in range(NB):
                    nc.tensor.matmul(
                        ps2[:, 512 * nb:512 * (nb + 1)],
                        lhsT=mm(seg_sb[:, pr, 2 * pr:2 * pr + 2]),
                        rhs=mm(sq[:, 512 * nb:512 * (nb + 1)]),
                        start=True, stop=True,
                    )
                ln_t = stp.tile([2, N], F32, name=f"ln{which}{pr}", tag="ln")
                nc.scalar.activation(
                    out=ln_t, in_=ps2, func=mybir.ActivationFunctionType.Ln,
                    bias=eps_t, scale=lsc,
                )
                inv = stp.tile([2, N], F32, name=f"inv{which}{pr}", tag="inv")
                nc.scalar.activation(
                    out=inv, in_=ln_t, func=mybir.ActivationFunctionType.Exp,
                    scale=-0.5,
                )
                sr = stp.tile([2, N], MMD, name=f"sigr{which}{pr}",
                              tag=f"sigr{which}", bufs=2)
                nc.vector.tensor_copy(out=sr, in_=inv)  # real cast: f32r rounds
                sig_r[pr][which] = sr

        def prescale(pr, which):
            tgt = qh_sb[pr] if which == "q" else kh_sb[pr]
            ps = psG.tile([128, N], F32, name=f"ps_b{which}{pr}", tag="g")
            for nb in range(NB):
                nc.tensor.matmul(
                    ps[:, 512 * nb:512 * (nb + 1)],
                    lhsT=mm(ind2[:, :]),
                    rhs=mm(sig_r[pr][which][:, 512 * nb:512 * (nb + 1)]),
                    start=True, stop=True,
                )
            nc.vector.tensor_mul(out=tgt, in0=tgt, in1=ps)

        def gen_v(mt):
            """V tile in bf16 with the ones column for softmax denominators."""
            ps = psG.tile([128, C], F32, name=f"ps_v{mt}", tag="g")
            for kc in range(CT):
                for vo, vn in ((0, 512), (512, 256)):
                    nc.tensor.matmul(
                        ps[:, vo:vo + vn],
                        lhsT=xtb_sb[kc][:, 128 * mt:128 * (mt + 1)],
                        rhs=wv_sb[kc][:, vo:vo + vn],
                        start=(kc == 0),
                        stop=(kc == CT - 1),
                    )
            t = persist.tile([128, H, D + 1], BF16, name=f"vsb{mt}", tag=f"v{mt}")
            nc.vector.memset(t, 1.0)
            nc.vector.tensor_copy(
                out=t[:, :, 0:D], in_=ps.rearrange("p (h d) -> p h d", h=H)
            )
            v_sb[mt] = t

        # ---- prefix: pair 0 (+ first V tile). gen-k borrows an (idle
        # until attention) psS slot so the k generation matmuls run while the
        # q stats/prescale chain serializes on the psG slot ----
        gen_qk(0, "q")
        gen_qk(0, "k", pstag="s")
        stats(0, "q")
        prescale(0, "q")
        stats(0, "k")
        prescale(0, "k")

        def epilogue(pr, nb, ot_ps):
            if ot_sb[pr] is None:
                # reuses qh[pr]'s slot — qh dies at this pair's last QK
                ot_sb[pr] = qkp.tile([128, N], BF16, name=f"ot{pr}", tag=f"qh{pr}")
            osb = []
            for j in range(2):
                t = epi.tile([D + 1, 512], F32, name=f"osb{pr}_{nb}_{j}", tag=f"osb{j}")
                nc.vector.tensor_copy(out=t, in_=ot_ps[j])
                osb.append(t)
            for j in range(2):
                h = 2 * pr + j
                rr = epi.tile([D + 1, 512], F32, name=f"rr{h}_{nb}", tag="rr")
                nc.vector.reciprocal_approx_fast(out=rr, in_=osb[j])
                dn_dram = dramp.tile([1, 512], F32, name=f"dnd{h}_{nb}", tag="dnd")
                nc.gpsimd.dma_start(out=dn_dram, in_=rr[D:D + 1, :])
                rbc = epi.tile([D, 512], F32, name=f"rbc{h}_{nb}", tag=f"rbc{j}")
                bcast_ap = bass.AP(
                    tensor=dn_dram.tensor, offset=dn_dram.offset,
                    ap=[[0, D]] + [list(a) for a in dn_dram.ap[1:]],
                )
                nc.gpsimd.dma_start(out=rbc, in_=bcast_ap)
                nsl = slice(512 * nb, 512 * (nb + 1))
                if j == 0:
                    nc.vector.tensor_mul(
                        out=ot_sb[pr][0:D, nsl], in0=osb[j][0:D, :], in1=rbc
                    )
                else:
                    tmpb = epi.tile([D, 512], BF16, name=f"tmpb{h}_{nb}", tag="tmpb")
                    nc.vector.tensor_mul(out=tmpb, in0=osb[j][0:D, :], in1=rbc)
                    nc.gpsimd.dma_start(out=ot_sb[pr][D:2 * D, nsl], in_=tmpb)

        pj_sb = [None] * NT

        _pjps = {}

        def proj_partial(nt, half):
            """Projection contribution of c-tiles 0..4 for token tile nt —
            runs as pair-5 filler (ot[0..4] are final by then); only the
            kc=5 term + an add remain after the last epilogue. Split in two
            filler-sized halves."""
            if half == 0:
                _pjps[nt] = psG.tile([128, C], F32, name=f"ps_pjp{nt}", tag="g")
            ps = _pjps[nt]
            kcs = (0, 1, 2) if half == 0 else (3, 4)
            for kc in kcs:
                for vo, vn in ((0, 512), (512, 256)):
                    nc.tensor.matmul(
                        ps[:, vo:vo + vn],
                        lhsT=ot_sb[kc][:, 128 * nt:128 * (nt + 1)],
                        rhs=wp_sb[kc][:, vo:vo + vn],
                        start=(kc == 0),
                        stop=(kc == CT - 2),
                    )
            if half == 1:
                t = persist.tile([128, C], F32, name=f"pjp{nt}", tag=f"pj{nt}")
                nc.vector.tensor_copy(out=t, in_=ps)
                pj_sb[nt] = t

        # Flat software-pipelined attention stream over all 96 (pr, nb, mt)
        # iterations with QK lookahead-1: per iteration emit QK(g+1), then
        # Exp(g), then this iteration's filler chunks (next-pair generation /
        # V tiles on the dedicated psG slot), then PV(g). The lookahead keeps
        # the ACT queue fed across iteration and pair boundaries; the filler
        # keeps the PE stream dense (HAM stays at full clock).
        iters = [(pr, nb, mt) for pr in range(NP) for nb in range(NB)
                 for mt in range(NT)]
        G = len(iters)

        sched = {}
        for mt in range(NT - 1):       # V[0] in prefix; V[mt+1] ready by PV(mt+1)
            sched[mt] = [lambda m=mt + 1: gen_v(m)]
        for nt in range(NT):
            for half in (0, 1):
                g = 5 * 16 + (2 * nt + half) % 16
                sched.setdefault(g, []).append(
                    lambda n=nt, hf=half: proj_partial(n, hf))
        for pr in range(NP - 1):
            chunks = [
                lambda p=pr + 1: gen_qk(p, "q"),
                lambda p=pr + 1: gen_qk(p, "k"),
                lambda p=pr + 1: stats(p, "q"),
                lambda p=pr + 1: stats(p, "k"),
                lambda p=pr + 1: prescale(p, "q"),
                lambda p=pr + 1: prescale(p, "k"),
            ]
            for off, ch in zip((1, 3, 5, 7, 9, 11), chunks):
                sched.setdefault(pr * 16 + off, []).append(ch)

        gen_v(0)
        wp_sb = load_w(wp, "wp", vp_pool, "vp", BF16, nc.sync)
        s_tiles = {}
        pt_tiles = {}
        ot_cur = [None, None]

        def emit_qk(g):
            pr, nb, mt = iters[g]
            s_ps = psS.tile([128, N], F32, name=f"ps_s{g}", tag="s")
            for j in range(2):
                nc.tensor.matmul(
                    s_ps[:, 512 * j:512 * (j + 1)],
                    lhsT=mm(kh_sb[pr][64 * j:64 * (j + 1), 128 * mt:128 * (mt + 1)]),
                    rhs=mm(qh_sb[pr][64 * j:64 * (j + 1), 512 * nb:512 * (nb + 1)]),
                    start=True, stop=True,
                )
            s_tiles[g] = s_ps

        emit_qk(0)
        for g in range(G):
            pr, nb, mt = iters[g]
            if g + 1 < G:
                emit_qk(g + 1)
            pt = ptp.tile([128, N], BF16, name=f"pt{g}", tag="pt")
            nc.scalar.activation(
                out=pt, in_=s_tiles.pop(g),
                func=mybir.ActivationFunctionType.Exp,
            )
            for ch in sched.get(g, ()):
                ch()
            if mt == 0:
                ot_cur = [
                    psOT.tile([D + 1, 512], F32, name=f"ps_ot{pr}_{nb}_{j}",
                              tag=f"ot{j}")
                    for j in range(2)
                ]
            for j in range(2):
                nc.tensor.matmul(
                    ot_cur[j][:, :],
                    lhsT=v_sb[mt][:, 2 * pr + j, :],
                    rhs=pt[:, 512 * j:512 * (j + 1)],
                    start=(mt == 0),
                    stop=(mt == NT - 1),
                )
            if mt == NT - 1:
                epilogue(pr, nb, ot_cur)

        # ---- projection finish: kc=5 term + add of the staged partials ----
        for nt in range(NT):
            ps = psS.tile([128, C], F32, name=f"ps_pj{nt}", tag="s")
            for vo, vn in ((0, 512), (512, 256)):
                nc.tensor.matmul(
                    ps[:, vo:vo + vn],
                    lhsT=ot_sb[CT - 1][:, 128 * nt:128 * (nt + 1)],
                    rhs=wp_sb[CT - 1][:, vo:vo + vn],
                    start=True, stop=True,
                )
            osb = work.tile([128, C], F32, name=f"osb{nt}", tag="sq")
            nc.vector.tensor_add(out=osb, in0=pj_sb[nt], in1=ps)
            eng = nc.sync if nt % 2 == 0 else nc.scalar
            eng.dma_start(out=out_ext[128 * nt:128 * (nt + 1), :], in_=osb)

    if compile_module:
        nc.compile()
    return nc


def prep_inputs(x, qkv_weight, proj_weight):
    import ml_dtypes
    x = np.asarray(x, dtype=np.float32)
    qkv_weight = np.asarray(qkv_weight, dtype=np.float32)
    proj_weight = np.asarray(proj_weight, dtype=np.float32)

    Wq = qkv_weight[0:C]
    Wk = qkv_weight[C:2 * C]
    Wv = qkv_weight[2 * C:3 * C]

    def center(W):
        Wc = W.reshape(H, D, C)
        Wc = Wc - Wc.mean(axis=1, keepdims=True)
        return Wc.reshape(C, C)

    wqcT = np.ascontiguousarray(center(Wq).T).astype(ml_dtypes.bfloat16)
    wkcT = np.ascontiguousarray(center(Wk).T).astype(ml_dtypes.bfloat16)
    wvT = np.ascontiguousarray(Wv.T).astype(ml_dtypes.bfloat16)
    wpT = np.ascontiguousarray(proj_weight.T).astype(ml_dtypes.bfloat16)

    seg = np.zeros((128, CT, H), ml_dtypes.bfloat16)
    for r in range(CT):
        for j in range(2):
            seg[64 * j:64 * (j + 1), r, 2 * r + j] = 1.0
    ind2 = np.zeros((2, 128), ml_dtypes.bfloat16)
    ind2[0, 0:64] = 1.0
    ind2[1, 64:128] = 1.0

    in_maps = []
    for b in range(x.shape[0]):
        xt = np.ascontiguousarray(x[b].T)
        in_maps.append(dict(
            xTb=xt.astype(ml_dtypes.bfloat16),
            wqcT=wqcT, wkcT=wkcT, wvT=wvT, wpT=wpT,
            seg=seg, ind2=ind2,
        ))
    return in_maps


_CACHE = {}


def kernel(x, qkv_weight, proj_weight):
    if "nc" not in _CACHE:
        _CACHE["nc"] = build()
    nc = _CACHE["nc"]
    in_maps = prep_inputs(x, qkv_weight, proj_weight)
    from concourse.bass_utils import run_bass_kernel_spmd
    res = run_bass_kernel_spmd(nc, in_maps, core_ids=list(range(len(in_maps))))
    out = np.stack([res.results[i]["out"] for i in range(len(in_maps))], axis=0)
    return out.astype(np.float32)


# revision 25
# speedup vs baseline: 1.1035x; 1.0359x over previous
"""Trainium2 Bass kernel for nn_AttentionCompiled (dense transformer attention).

B=8, N=1024, C=768, H=12 heads, D=64. Per-head LayerNorm on q/k, softmax
attention, output projection. Pure data parallelism: one batch element per
NeuronCore, weights replicated, no collectives.

Math folding:
 - LN centering folded into Wq/Wk (CPU-side).
 - 1/sigma_q (with the 1/sqrt(D) scale) folded into q and 1/sigma_k into k via
   PE-broadcast matmul + DVE multiply, so Exp runs with scale=1 and both
   512-wide query blocks batch into one FD=1024 ACTIVATE.
 - 1/sigma = exp(-0.5*ln(scale*sumsq + eps)): ln and exp share ONE ACT table
   set (natural_log_exp_and_others), so per-pair stats interleave with
   attention exps with zero table reloads. build() pins that choice by
   emptying the single-function exp/ln sets in the table map (index-preserving
   — act_func_set_id indexes act_info.json).
 - Softmax denominators from an appended ones-column on V (row 64 of O^T).
 - |scores| <= 8 (Cauchy-Schwarz on LN'd vectors): exp needs no max-subtract.

Precision split: q/k generation and QK^T run in f32r (score exponents are
error-sensitive); the V path (x_bf16 @ Wv), P (exp output), O^T and the
projection run in bf16 with fp32 PSUM accumulation (gate is 2e-2).

Perf structure (v3):
 - Per-pair software pipeline: pair p's ACT-paced attention stream carries
   pair p+1's generation/stats/prescale matmuls (and, for pair 0, the whole
   V generation) as PE filler, so the PE instruction stream stays dense —
   idle PE windows re-engage the HAM clock gate (K=4/8, half clock), which
   is what capped v1/v2.
 - Loop order pr -> nb -> mt; score tile [128,1024] holds both heads
   (row-group concurrent QK); one Exp ACTIVATE per tile (FD=1024).
 - PSUM: 2x score tiles (4 banks) + 2x2 O^T accumulators [65,512] (4 banks,
   double-buffered so the next pair's PV never waits the epilogue copies).
 - Epilogue per (pr, nb): copy accumulators out, reciprocal_approx_fast
   (full-tile: the custom DVE op mishandles base_partition != 0), DMA
   row-broadcast via DRAM bounce, DVE normalize into bf16 O^T.
"""

import sys
import numpy as np
from contextlib import ExitStack

if "/opt/trn_rl_repo" not in sys.path:
    sys.path.insert(0, "/opt/trn_rl_repo")

import concourse.bass as bass
import concourse.bacc as bacc
import concourse.tile as tile
from concourse import mybir

F32 = mybir.dt.float32
F32R = mybir.dt.float32r
BF16 = mybir.dt.bfloat16

N = 1024
C = 768
H = 12
D = 64
NT = N // 128
CT = C // 128
NB = N // 512
NP = H // 2
EPS = 1e-5

USE_F32R = True


def _filtered_act_tables(arch):
    import concourse.hw_specs as hw_specs
    tabs = dict(hw_specs.get_activation_tables(arch))
    # empty them (never match) rather than delete: act_func_set_id is the
    # INDEX into this ordered dict and must stay aligned with act_info.json
    for k in ("exp_and_others", "natural_log", "exp_and_friends"):
        tabs[k] = set()
    return tabs


def build(use_f32r: bool = USE_F32R, compile_module: bool = True) -> bass.Bass:
    bacc.get_activation_tables = _filtered_act_tables
    nc = bacc.Bacc()

    xTb = nc.declare_dram_parameter("xTb", [C, N], BF16, isOutput=False)
    wq = nc.declare_dram_parameter("wqcT", [C, C], BF16, isOutput=False)
    wk = nc.declare_dram_parameter("wkcT", [C, C], BF16, isOutput=False)
    wv = nc.declare_dram_parameter("wvT", [C, C], BF16, isOutput=False)
    wp = nc.declare_dram_parameter("wpT", [C, C], BF16, isOutput=False)
    seg = nc.declare_dram_parameter("seg", [128, CT, H], BF16, isOutput=False)
    ind2d = nc.declare_dram_parameter("ind2", [2, 128], BF16, isOutput=False)
    out_ext = nc.declare_dram_parameter("out", [N, C], F32, isOutput=True)

    MMD = BF16

    def mm(ap):
        return ap

    with tile.TileContext(nc) as tc, ExitStack() as ctx:
        xo_pool = ctx.enter_context(tc.tile_pool(name="xo", bufs=6))
        persist = ctx.enter_context(tc.tile_pool(name="persist", bufs=1))
        vp_pool = ctx.enter_context(tc.tile_pool(name="vp", bufs=6))
        work = ctx.enter_context(tc.tile_pool(name="work", bufs=2))
        ptp = ctx.enter_context(tc.tile_pool(name="ptp", bufs=3))
        epi = ctx.enter_context(tc.tile_pool(name="epi", bufs=1))
        stp = ctx.enter_context(tc.tile_pool(name="stp", bufs=1))
        qkp = ctx.enter_context(tc.tile_pool(name="qkp", bufs=1))
        rows = ctx.enter_context(tc.tile_pool(name="rows", bufs=1))
        dramp = ctx.enter_context(tc.tile_pool(name="dramp", bufs=2, space="DRAM"))
        psS = ctx.enter_context(tc.tile_pool(name="psS", bufs=2, space="PSUM"))
        psG = ctx.enter_context(tc.tile_pool(name="psG", bufs=1, space="PSUM"))
        psOT = ctx.enter_context(tc.tile_pool(name="psOT", bufs=1, space="PSUM"))

        # ---- loads: spread across engine DMA queues so the prefix isn't
        # serialized on one queue, and interleave (wq[kc], xt[kc]) so the
        # first gen matmuls can start after the first c-tile lands ----
        def load_w(dram, nm, pool, tag, dt, eng, per_tile_tags=False):
            tiles = []
            for r in range(CT):
                t = pool.tile(
                    [128, C], dt, name=f"{nm}{r}",
                    tag=(f"{tag}{r}" if per_tile_tags else tag),
                )
                eng.dma_start(out=t, in_=mm(dram[128 * r:128 * (r + 1), :]))
                tiles.append(t)
            return tiles

        xtb_sb = []
        wq_sb = load_w(wq, "wq", persist, "wq", BF16, nc.scalar, per_tile_tags=True)
        for r in range(CT):
            t = xo_pool.tile([128, N], BF16, name=f"xtb{r}", tag="xb")
            nc.sync.dma_start(out=t, in_=xTb[128 * r:128 * (r + 1), :])
            xtb_sb.append(t)
        seg_sb = rows.tile([128, CT, H], MMD, name="seg", tag="seg")
        nc.scalar.dma_start(out=seg_sb, in_=mm(seg[:, :, :]))
        ind2 = rows.tile([2, 128], MMD, name="ind2", tag="ind2")
        nc.scalar.dma_start(out=ind2, in_=mm(ind2d[:, :]))
        wk_sb = load_w(wk, "wk", persist, "wk", BF16, nc.gpsimd, per_tile_tags=True)
        wv_sb = load_w(wv, "wv", vp_pool, "vp", BF16, nc.gpsimd)
        xt_sb = xtb_sb

        epsq2 = rows.tile([2, 1], F32, name="epsq2", tag="epsq2")
        nc.vector.memset(epsq2, float(D) * EPS)
        epsk2 = rows.tile([2, 1], F32, name="epsk2", tag="epsk2")
        nc.vector.memset(epsk2, EPS)

        qh_sb = [None] * NP
        kh_sb = [None] * NP
        sig_r = [None] * NP
        v_sb = [None] * NT
        ot_sb = [None] * CT

        def gen_qk(pr, which, pstag="g"):
            wt = wq_sb if which == "q" else wk_sb
            lst = qh_sb if which == "q" else kh_sb
            pool = psG if pstag == "g" else psS
            ps = pool.tile([128, N], F32, name=f"ps_{which}{pr}", tag=pstag)
            for kc in range(CT):
                for nb in range(NB):
                    nc.tensor.matmul(
                        ps[:, 512 * nb:512 * (nb + 1)],
                        lhsT=mm(wt[kc][:, 128 * pr:128 * (pr + 1)]),
                        rhs=mm(xt_sb[kc][:, 512 * nb:512 * (nb + 1)]),
                        start=(kc == 0),
                        stop=(kc == CT - 1),
                    )
            t = qkp.tile([128, N], MMD, name=f"{which}h{pr}", tag=f"{which}h{pr}")
            nc.vector.tensor_copy(out=t, in_=ps)
            lst[pr] = t

        def stats(pr, which):
            """1/sigma for one of q/k of pair pr via exp(-0.5*ln(...)) —
            same ACT table set as the attention Exp."""
            if sig_r[pr] is None:
                sig_r[pr] = {}
            for which, src, eps_t, lsc in (
                (("q", qh_sb[pr], epsq2, 1.0),) if which == "q"
                else (("k", kh_sb[pr], epsk2, 1.0 / D),)
            ):
                sq = work.tile([128, N], MMD, name=f"sq_{which}{pr}", tag="sq")
                nc.vector.tensor_mul(out=sq, in0=src, in1=src)
                ps2 = psG.tile([2, N], F32, name=f"ps_st{which}{pr}", tag="g")
                for nb in range(NB):
                    nc.tensor.matmul(
                        ps2[:, 512 * nb:512 * (nb + 1)],
                        lhsT=mm(seg_sb[:, pr, 2 * pr:2 * pr + 2]),
                        rhs=mm(sq[:, 512 * nb:512 * (nb + 1)]),
                        start=True, stop=True,
                    )
                ln_t = stp.tile([2, N], F32, name=f"ln{which}{pr}", tag="ln")
                nc.scalar.activation(
                    out=ln_t, in_=ps2, func=mybir.ActivationFunctionType.Ln,
                    bias=eps_t, scale=lsc,
                )
                inv = stp.tile([2, N], F32, name=f"inv{which}{pr}", tag="inv")
                nc.scalar.activation(
                    out=inv, in_=ln_t, func=mybir.ActivationFunctionType.Exp,
                    scale=-0.5,
                )
                sr = stp.tile([2, N], MMD, name=f"sigr{which}{pr}",
                              tag=f"sigr{which}", bufs=2)
                nc.vector.tensor_copy(out=sr, in_=inv)  # real cast: f32r rounds
                sig_r[pr][which] = sr

        def prescale(pr, which):
            tgt = qh_sb[pr] if which == "q" else kh_sb[pr]
            ps = psG.tile([128, N], F32, name=f"ps_b{which}{pr}", tag="g")
            for nb in range(NB):
                nc.tensor.matmul(
                    ps[:, 512 * nb:512 * (nb + 1)],
                    lhsT=mm(ind2[:, :]),
                    rhs=mm(sig_r[pr][which][:, 512 * nb:512 * (nb + 1)]),
                    start=True, stop=True,
                )
            nc.vector.tensor_mul(out=tgt, in0=tgt, in1=ps)

        def gen_v(mt, pstag="g"):
            """V tile in bf16 with the ones column for softmax denominators."""
            pool = psG if pstag == "g" else psS
            ps = pool.tile([128, C], F32, name=f"ps_v{mt}", tag=pstag)
            for kc in range(CT):
                for vo, vn in ((0, 512), (512, 256)):
                    nc.tensor.matmul(
                        ps[:, vo:vo + vn],
                        lhsT=xtb_sb[kc][:, 128 * mt:128 * (mt + 1)],
                        rhs=wv_sb[kc][:, vo:vo + vn],
                        start=(kc == 0),
                        stop=(kc == CT - 1),
                    )
            t = persist.tile([128, H, D + 1], BF16, name=f"vsb{mt}", tag=f"v{mt}")
            nc.vector.memset(t, 1.0)
            nc.vector.tensor_copy(
                out=t[:, :, 0:D], in_=ps.rearrange("p (h d) -> p h d", h=H)
            )
            v_sb[mt] = t

        # ---- prefix: pair 0 (+ first V tile). gen-k borrows an (idle
        # until attention) psS slot so the k generation matmuls run while the
        # q stats/prescale chain serializes on the psG slot ----
        gen_qk(0, "q")
        gen_qk(0, "k", pstag="s")
        for mt in range(NT):
            gen_v(mt, pstag="s")   # fills the PE while the stats chain runs
        stats(0, "q")
        prescale(0, "q")
        stats(0, "k")
        prescale(0, "k")

        def epilogue(pr, nb, ot_ps):
            if ot_sb[pr] is None:
                # reuses qh[pr]'s slot — qh dies at this pair's last QK
                ot_sb[pr] = qkp.tile([128, N], BF16, name=f"ot{pr}", tag=f"qh{pr}")
            osb = []
            for j in range(2):
                t = epi.tile([D + 1, 512], F32, name=f"osb{pr}_{nb}_{j}", tag=f"osb{j}")
                nc.vector.tensor_copy(out=t, in_=ot_ps[j])
                osb.append(t)
            for j in range(2):
                h = 2 * pr + j
                rr = epi.tile([D + 1, 512], F32, name=f"rr{h}_{nb}", tag="rr")
                nc.vector.reciprocal_approx_fast(out=rr, in_=osb[j])
                dn_dram = dramp.tile([1, 512], F32, name=f"dnd{h}_{nb}", tag="dnd")
                nc.gpsimd.dma_start(out=dn_dram, in_=rr[D:D + 1, :])
                rbc = epi.tile([D, 512], F32, name=f"rbc{h}_{nb}", tag=f"rbc{j}")
                bcast_ap = bass.AP(
                    tensor=dn_dram.tensor, offset=dn_dram.offset,
                    ap=[[0, D]] + [list(a) for a in dn_dram.ap[1:]],
                )
                nc.gpsimd.dma_start(out=rbc, in_=bcast_ap)
                nsl = slice(512 * nb, 512 * (nb + 1))
                if j == 0:
                    nc.vector.tensor_mul(
                        out=ot_sb[pr][0:D, nsl], in0=osb[j][0:D, :], in1=rbc
                    )
                else:
                    tmpb = epi.tile([D, 512], BF16, name=f"tmpb{h}_{nb}", tag="tmpb")
                    nc.vector.tensor_mul(out=tmpb, in0=osb[j][0:D, :], in1=rbc)
                    nc.gpsimd.dma_start(out=ot_sb[pr][D:2 * D, nsl], in_=tmpb)

        pj_sb = [None] * NT

        _pjps = {}

        def proj_partial(nt, half):
            """Projection contribution of c-tiles 0..4 for token tile nt —
            runs as pair-5 filler (ot[0..4] are final by then); only the
            kc=5 term + an add remain after the last epilogue. Split in two
            filler-sized halves."""
            if half == 0:
                _pjps[nt] = psG.tile([128, C], F32, name=f"ps_pjp{nt}", tag="g")
            ps = _pjps[nt]
            kcs = (0, 1, 2) if half == 0 else (3, 4)
            for kc in kcs:
                for vo, vn in ((0, 512), (512, 256)):
                    nc.tensor.matmul(
                        ps[:, vo:vo + vn],
                        lhsT=ot_sb[kc][:, 128 * nt:128 * (nt + 1)],
                        rhs=wp_sb[kc][:, vo:vo + vn],
                        start=(kc == 0),
                        stop=(kc == CT - 2),
                    )
            if half == 1:
                t = persist.tile([128, C], F32, name=f"pjp{nt}", tag=f"pj{nt}")
                nc.vector.tensor_copy(out=t, in_=ps)
                pj_sb[nt] = t

        # Flat software-pipelined attention stream over all 96 (pr, nb, mt)
        # iterations with QK lookahead-1: per iteration emit QK(g+1), then
        # Exp(g), then this iteration's filler chunks (next-pair generation /
        # V tiles on the dedicated psG slot), then PV(g). The lookahead keeps
        # the ACT queue fed across iteration and pair boundaries; the filler
        # keeps the PE stream dense (HAM stays at full clock).
        iters = [(pr, nb, mt) for pr in range(NP) for nb in range(NB)
                 for mt in range(NT)]
        G = len(iters)

        sched = {}
        for nt in range(NT):
            for half in (0, 1):
                g = 5 * 16 + (2 * nt + half) % 16
                sched.setdefault(g, []).append(
                    lambda n=nt, hf=half: proj_partial(n, hf))
        for pr in range(NP - 1):
            chunks = [
                lambda p=pr + 1: gen_qk(p, "q"),
                lambda p=pr + 1: gen_qk(p, "k"),
                lambda p=pr + 1: stats(p, "q"),
                lambda p=pr + 1: stats(p, "k"),
                lambda p=pr + 1: prescale(p, "q"),
                lambda p=pr + 1: prescale(p, "k"),
            ]
            for off, ch in zip((1, 3, 5, 7, 9, 11), chunks):
                sched.setdefault(pr * 16 + off, []).append(ch)

        wp_sb = load_w(wp, "wp", vp_pool, "vp", BF16, nc.sync)
        s_tiles = {}
        pt_tiles = {}
        ot_cur = [None, None]

        def emit_qk(g):
            pr, nb, mt = iters[g]
            s_ps = psS.tile([128, N], F32, name=f"ps_s{g}", tag="s")
            for j in range(2):
                nc.tensor.matmul(
                    s_ps[:, 512 * j:512 * (j + 1)],
                    lhsT=mm(kh_sb[pr][64 * j:64 * (j + 1), 128 * mt:128 * (mt + 1)]),
                    rhs=mm(qh_sb[pr][64 * j:64 * (j + 1), 512 * nb:512 * (nb + 1)]),
                    start=True, stop=True,
                )
            s_tiles[g] = s_ps

        emit_qk(0)
        for g in range(G):
            pr, nb, mt = iters[g]
            if g + 1 < G:
                emit_qk(g + 1)
            pt = ptp.tile([128, N], BF16, name=f"pt{g}", tag="pt")
            nc.scalar.activation(
                out=pt, in_=s_tiles.pop(g),
                func=mybir.ActivationFunctionType.Exp,
            )
            for ch in sched.get(g, ()):
                ch()
            if mt == 0:
                ot_cur = [
                    psOT.tile([D + 1, 512], F32, name=f"ps_ot{pr}_{nb}_{j}",
                              tag=f"ot{j}")
                    for j in range(2)
                ]
            for j in range(2):
                nc.tensor.matmul(
                    ot_cur[j][:, :],
                    lhsT=v_sb[mt][:, 2 * pr + j, :],
                    rhs=pt[:, 512 * j:512 * (j + 1)],
                    start=(mt == 0),
                    stop=(mt == NT - 1),
                )
            if mt == NT - 1:
                epilogue(pr, nb, ot_cur)

        # ---- projection finish: kc=5 term + add of the staged partials ----
        for nt in range(NT):
            ps = psS.tile([128, C], F32, name=f"ps_pj{nt}", tag="s")
            for vo, vn in ((0, 512), (512, 256)):
                nc.tensor.matmul(
                    ps[:, vo:vo + vn],
                    lhsT=ot_sb[CT - 1][:, 128 * nt:128 * (nt + 1)],
                    rhs=wp_sb[CT - 1][:, vo:vo + vn],
                    start=True, stop=True,
                )
            osb = work.tile([128, C], F32, name=f"osb{nt}", tag="sq")
            nc.vector.tensor_add(out=osb, in0=pj_sb[nt], in1=ps)
            eng = nc.sync if nt % 2 == 0 else nc.scalar
            eng.dma_start(out=out_ext[128 * nt:128 * (nt + 1), :], in_=osb)

    if compile_module:
        nc.compile()
    return nc


def prep_inputs(x, qkv_weight, proj_weight):
    import ml_dtypes
    x = np.asarray(x, dtype=np.float32)
    qkv_weight = np.asarray(qkv_weight, dtype=np.float32)
    proj_weight = np.asarray(proj_weight, dtype=np.float32)

    Wq = qkv_weight[0:C]
    Wk = qkv_weight[C:2 * C]
    Wv = qkv_weight[2 * C:3 * C]

    def center(W):
        Wc = W.reshape(H, D, C)
        Wc = Wc - Wc.mean(axis=1, keepdims=True)
        return Wc.reshape(C, C)

    wqcT = np.ascontiguousarray(center(Wq).T).astype(ml_dtypes.bfloat16)
    wkcT = np.ascontiguousarray(center(Wk).T).astype(ml_dtypes.bfloat16)
    wvT = np.ascontiguousarray(Wv.T).astype(ml_dtypes.bfloat16)
    wpT = np.ascontiguousarray(proj_weight.T).astype(ml_dtypes.bfloat16)

    seg = np.zeros((128, CT, H), ml_dtypes.bfloat16)
    for r in range(CT):
        for j in range(2):
            seg[64 * j:64 * (j + 1), r, 2 * r + j] = 1.0
    ind2 = np.zeros((2, 128), ml_dtypes.bfloat16)
    ind2[0, 0:64] = 1.0
    ind2[1, 64:128] = 1.0

    in_maps = []
    for b in range(x.shape[0]):
        xt = np.ascontiguousarray(x[b].T)
        in_maps.append(dict(
            xTb=xt.astype(ml_dtypes.bfloat16),
            wqcT=wqcT, wkcT=wkcT, wvT=wvT, wpT=wpT,
            seg=seg, ind2=ind2,
        ))
    return in_maps


_CACHE = {}


def kernel(x, qkv_weight, proj_weight):
    if "nc" not in _CACHE:
        _CACHE["nc"] = build()
    nc = _CACHE["nc"]
    in_maps = prep_inputs(x, qkv_weight, proj_weight)
    from concourse.bass_utils import run_bass_kernel_spmd
    res = run_bass_kernel_spmd(nc, in_maps, core_ids=list(range(len(in_maps))))
    out = np.stack([res.results[i]["out"] for i in range(len(in_maps))], axis=0)
    return out.astype(np.float32)
